# revision 22
# baseline (speedup 1.0000x reference)
"""Trainium2 Bass kernel for the CliffordPC problem.

Self-contained: takes FULL inputs, shards batch across 8 NeuronCores,
runs all 20 predictive-coding iterations on-chip, returns FULL outputs.

Math. The geometric-product "linear layer"
    out[n,i,c] = sum_{j,a,b} x[n,j,a] * W[i,j,b] * C[a,b,c]
with the Cl(3,0) Cayley table C (one nonzero per (a,b): C[a,b,a^b]=s(a,b))
is exactly a dense matmul over flattened (feature, blade) axes:
    out[n, i*8+c] = sum_{j,a} x[n, j*8+a] * M[(j,a),(i,c)],
    M[(j,a),(i,c)] = s(a, a^c) * W[i, j, a^c].
All operators are precomputed on the host as dense matrices in float32r
(fp32 with mantissa RNE-rounded to 11 bits -> 4x matmul throughput on the
PE array at free-dim >= 256, fp32-exact accumulation in PSUM).

Key fusion: the layer-0 error e0 = x - h1@MA only feeds g1 = e0@MB, so
    g1 = x@MB - h1@(MA@MB) = XB - h1@MAB
where XB = x@MB is iteration-invariant (x never changes; computed once on
chip) and MAB = MA@MB is precomputed on the host. This cuts the dominant
per-iteration work from two 2048-wide matmuls to one 1024-wide matmul
(~2.4x FLOP reduction).
"""

import sys
import os

for _p in ("/opt/trn_rl_repo",):
    if _p not in sys.path and os.path.isdir(_p):
        sys.path.insert(0, _p)

import numpy as np
from contextlib import ExitStack

import concourse.bass as bass
import concourse.tile as tile
from concourse import bacc, mybir
from concourse.bass_utils import run_bass_kernel_spmd

# ---- problem constants (hardcoded per contract) ----
B = 4096
D0, D1, D2 = 256, 128, 64       # layer dims
NB = 8                           # blades (Cl(3))
F0, F1, F2 = D0 * NB, D1 * NB, D2 * NB   # 2048, 1024, 512
N_ITER = 20
ALPHA = 0.01
N_CORES = 8
B_CORE = B // N_CORES            # 512
NT = 512                         # batch-tile width == B_CORE (one tile/core)
P = 128

K0, K1, K2 = F0 // P, F1 // P, F2 // P   # 16, 8, 4 partition-chunks

f32 = mybir.dt.float32
f32r = mybir.dt.float32r


# ---------------- host-side math ----------------

def _cayley_sign():
    """s[a,b] for Cl(3,0,0) with metric (1,1,1): e_a e_b = s[a,b] e_{a^b}."""
    s = np.zeros((NB, NB), np.float32)
    for a in range(NB):
        for b in range(NB):
            cnt = 0
            aa = a >> 1
            while aa:
                cnt += bin(aa & b).count("1")
                aa >>= 1
            s[a, b] = -1.0 if (cnt & 1) else 1.0
    return s


def _rev_signs():
    k = np.array([bin(i).count("1") for i in range(NB)])
    return ((-1.0) ** (k * (k - 1) // 2)).astype(np.float32)


def _build_mats(W1, W2):
    """Expanded dense matrices for the four geometric-product operators."""
    S = _cayley_sign()                       # [a,b]
    REV = _rev_signs()                       # [b]
    a_idx = np.arange(NB)
    c_idx = np.arange(NB)
    Bx = a_idx[:, None] ^ c_idx[None, :]     # b = a^c, [a,c]
    Sac = S[a_idx[:, None], Bx]              # s(a, a^c), [a,c]
    SacR = Sac * REV[Bx]                     # s(a, a^c)*REV[a^c], [a,c]

    # MA[(j,a),(i,c)] = Sac[a,c] * W1[i,j,a^c]         -> [F1, F0]
    W1g = W1[:, :, Bx]                       # [i,j,a,c]
    MA = (Sac[None, None] * W1g).transpose(1, 2, 0, 3).reshape(F1, F0)

    # MB[(j,a),(i,c)] = SacR[a,c] * W1[j,i,a^c]        -> [F0, F1]
    MB = (SacR[None, None] * W1g).transpose(0, 2, 1, 3).reshape(F0, F1)

    # MC[(j,a),(i,c)] = Sac[a,c] * W2[i,j,a^c]         -> [F2, F1]
    W2g = W2[:, :, Bx]                       # [i,j,a,c]
    MC = (Sac[None, None] * W2g).transpose(1, 2, 0, 3).reshape(F2, F1)

    # MD[(j,a),(i,c)] = SacR[a,c] * W2[j,i,a^c]        -> [F1, F2]
    MD = (SacR[None, None] * W2g).transpose(0, 2, 1, 3).reshape(F1, F2)

    return MA, MB, MC, MD


def _round_fp32r(a):
    """RNE-round fp32 to fp32r (drop 12 low mantissa bits) - matches HW cast."""
    b = np.ascontiguousarray(a, dtype=np.float32).view(np.uint32)
    r = b + np.uint32(0x7FF) + ((b >> np.uint32(12)) & np.uint32(1))
    r &= np.uint32(0xFFFFF000)
    return r.view(np.float32)


def _chunked(mat, k_tiles):
    """[K, N] row-major -> [k_tiles, P, N] (k-chunk, partition, cols)."""
    K, N = mat.shape
    assert K == k_tiles * P
    return np.ascontiguousarray(mat.reshape(k_tiles, P, N))


# ---------------- device kernel ----------------

def _build_bass():
    n_iter = int(os.environ.get("KERNEL_N_ITER", N_ITER))
    nc = bacc.Bacc("TRN2", target_bir_lowering=False, debug=False)

    XB_d = nc.dram_tensor("XBT", [K1, P, B_CORE], f32, kind="ExternalInput").ap()
    h1_d = nc.dram_tensor("h1T", [K1, P, B_CORE], f32r, kind="ExternalInput").ap()
    h2_d = nc.dram_tensor("h2T", [K2, P, B_CORE], f32r, kind="ExternalInput").ap()
    MAB_d = nc.dram_tensor("MAB", [K1, P, F1], f32r, kind="ExternalInput").ap()
    MC_d = nc.dram_tensor("MC", [K2, P, F1], f32r, kind="ExternalInput").ap()
    MD_d = nc.dram_tensor("MD", [K1, P, F2], f32r, kind="ExternalInput").ap()
    h1o_d = nc.dram_tensor("h1o", [K1, P, B_CORE], f32, kind="ExternalOutput").ap()
    h2o_d = nc.dram_tensor("h2o", [K2, P, B_CORE], f32, kind="ExternalOutput").ap()

    AMIN, AMAX = mybir.AluOpType.min, mybir.AluOpType.max
    TW = 256          # per-batch-tile width; two tiles are interleaved so
    NTL = 2           # one tile's matmuls hide the other's vector chains

    with tile.TileContext(nc) as tc, ExitStack() as ctx:
        wpool = ctx.enter_context(tc.tile_pool(name="weights", bufs=1))
        state = ctx.enter_context(tc.tile_pool(name="state", bufs=1))
        tpool = ctx.enter_context(tc.tile_pool(name="tmps", bufs=3))
        # one accumulation group per PSUM bank (HW start=True clears the
        # whole bank's has_written bits): 3 + 3 + 2 = 8 banks
        ps_acc = ctx.enter_context(tc.tile_pool(name="ps_acc", bufs=3, space="PSUM"))
        ps_s = ctx.enter_context(tc.tile_pool(name="ps_s", bufs=3, space="PSUM"))
        ps_g2 = ctx.enter_context(tc.tile_pool(name="ps_g2", bufs=2, space="PSUM"))

        # resident weights
        MAB_s = wpool.tile([P, K1, F1], f32r, tag="MAB")
        MC_s = wpool.tile([P, K2, F1], f32r, tag="MC")
        MD_s = wpool.tile([P, K1, F2], f32r, tag="MD")

        # per-batch-tile states: fp32 masters + f32r matmul copies
        h1m = [state.tile([P, K1, TW], f32, tag=f"h1m{t}", name=f"h1m{t}") for t in range(NTL)]
        h1r = [state.tile([P, K1, TW], f32r, tag=f"h1r{t}", name=f"h1r{t}") for t in range(NTL)]
        h2m = [state.tile([P, K2, TW], f32, tag=f"h2m{t}", name=f"h2m{t}") for t in range(NTL)]
        h2r = [state.tile([P, K2, TW], f32r, tag=f"h2r{t}", name=f"h2r{t}") for t in range(NTL)]
        m_t = [state.tile([P, K1, TW], f32, tag=f"m{t}", name=f"m{t}") for t in range(NTL)]
        XB_t = [state.tile([P, K1, TW], f32, tag=f"XB{t}", name=f"XB{t}") for t in range(NTL)]
        e1_t = [state.tile([P, K1, TW], f32r, tag=f"e1{t}", name=f"e1{t}") for t in range(NTL)]

        def bsl(t):
            return bass.ts(t, TW)

        # Cold-start DMA across queues; A(0) needs h2r+MC first.
        for t in range(NTL):
            nc.sync.dma_start(h2r[t][:], h2_d.rearrange("k p b -> p k b")[:, :, bsl(t)])
            nc.sync.dma_start(
                h2m[t][:], h2_d.bitcast(f32).rearrange("k p b -> p k b")[:, :, bsl(t)]
            )
        nc.gpsimd.dma_start(MC_s[:], MC_d.rearrange("k p n -> p k n"))
        for t in range(NTL):
            nc.sync.dma_start(h1r[t][:], h1_d.rearrange("k p b -> p k b")[:, :, bsl(t)])
            nc.sync.dma_start(
                h1m[t][:], h1_d.bitcast(f32).rearrange("k p b -> p k b")[:, :, bsl(t)]
            )
        for k in range(K1):
            nc.scalar.dma_start(
                MAB_s[:, k, :], MAB_d.rearrange("k p n -> p k n")[:, k, :]
            )
        for t in range(NTL):
            nc.sync.dma_start(XB_t[t][:], XB_d.rearrange("k p b -> p k b")[:, :, bsl(t)])
        nc.gpsimd.dma_start(MD_s[:], MD_d.rearrange("k p n -> p k n"))

        def phase_A(t, it):
            # p2 = MC.h2 ; m = h1 - p2 (fused eviction)
            for g in range(K1):
                ps = ps_acc.tile([P, TW], f32, tag="acc", name=f"p2_{it}_{t}_{g}")
                for k in range(K2):
                    nc.tensor.matmul(
                        ps[:],
                        MC_s[:, k, bass.ts(g, P)],
                        h2r[t][:, k, :],
                        start=(k == 0),
                        stop=(k == K2 - 1),
                    )
                nc.vector.tensor_sub(m_t[t][:, g, :], h1m[t][:, g, :], ps[:])

        for t in range(NTL):
            phase_A(t, 0)

        for it in range(n_iter):
            # ---- C1 (tile-interleaved): S = MAB.h1 ;
            #      c2 = ALPHA*clip((m - XB) + S) ; e1 = m - c2 ;
            #      h1 -= c2 ; h1r = round(h1) ----
            for g in range(K1):
                for t in range(NTL):
                    sps = ps_s.tile([P, TW], f32, tag="s", name=f"s_{it}_{t}_{g}")
                    for k in range(K1):
                        nc.tensor.matmul(
                            sps[:],
                            MAB_s[:, k, bass.ts(g, P)],
                            h1r[t][:, k, :],
                            start=(k == 0),
                            stop=(k == K1 - 1),
                        )
                    tt = tpool.tile([P, TW], f32, tag="t")
                    nc.vector.tensor_sub(tt[:], m_t[t][:, g, :], XB_t[t][:, g, :])
                    nc.vector.tensor_add(tt[:], tt[:], sps[:])
                    t2 = tpool.tile([P, TW], f32, tag="t2")
                    # c2 = ALPHA*clip(t,-1,1) == clip(ALPHA*t, -ALPHA, ALPHA),
                    # kept on DVE: the ACT scale-multiply is internally
                    # low-precision and biased the h1 trajectory
                    nc.vector.tensor_scalar(t2[:], tt[:], ALPHA, ALPHA, mybir.AluOpType.mult, AMIN)
                    nc.vector.tensor_scalar_max(t2[:], t2[:], -ALPHA)
                    nc.vector.tensor_sub(e1_t[t][:, g, :], m_t[t][:, g, :], t2[:])
                    # master update off the DVE critical path
                    nc.gpsimd.tensor_sub(h1m[t][:, g, :], h1m[t][:, g, :], t2[:])
                    nc.scalar.mul(h1r[t][:, g, :], h1m[t][:, g, :], 1.0)

            # ---- C2+D per tile: g2 = MD.e1 (q-outer, one bank per group) ;
            #      h2 += ALPHA*clip(g2) ; h2r = round(h2) ----
            for t in range(NTL):
                for q in range(K2):
                    g2ps = ps_g2.tile([P, TW], f32, tag="g2", name=f"g2_{it}_{t}_{q}")
                    for g in range(K1):
                        nc.tensor.matmul(
                            g2ps[:],
                            MD_s[:, g, bass.ts(q, P)],
                            e1_t[t][:, g, :],
                            start=(g == 0),
                            stop=(g == K1 - 1),
                        )
                    u = tpool.tile([P, TW], f32, tag="t", name=f"u_{it}_{t}_{q}")
                    nc.vector.tensor_scalar(u[:], g2ps[:], 1.0, -1.0, AMIN, AMAX)
                    nc.scalar.mul(u[:], u[:], ALPHA)
                    nc.gpsimd.tensor_add(h2m[t][:, q, :], h2m[t][:, q, :], u[:])
                    nc.scalar.mul(h2r[t][:, q, :], h2m[t][:, q, :], 1.0)

            if it + 1 < n_iter:
                for t in range(NTL):
                    phase_A(t, it + 1)

        for t in range(NTL):
            nc.gpsimd.dma_start(
                h1o_d.rearrange("k p b -> p k b")[:, :, bsl(t)], h1m[t][:]
            )
            nc.gpsimd.dma_start(
                h2o_d.rearrange("k p b -> p k b")[:, :, bsl(t)], h2m[t][:]
            )

    nc.compile()
    return nc


_NC_CACHE = None


def _get_nc():
    global _NC_CACHE
    if _NC_CACHE is None:
        _NC_CACHE = _build_bass()
    return _NC_CACHE


def kernel(x, W1, W2, h1, h2, _trace=False):
    x = np.asarray(x, dtype=np.float32)
    W1 = np.asarray(W1, dtype=np.float32)
    W2 = np.asarray(W2, dtype=np.float32)
    h1 = np.asarray(h1, dtype=np.float32)
    h2 = np.asarray(h2, dtype=np.float32)

    MA, MB, MC, MD = _build_mats(W1, W2)
    MB64 = MB.astype(np.float64)
    MAB = (MA.astype(np.float64) @ MB64).astype(np.float32)
    MAB_h = _chunked(_round_fp32r(MAB), K1)
    MC_h = _chunked(_round_fp32r(MC), K2)
    MD_h = _chunked(_round_fp32r(MD), K1)
    # XB = x @ MB on host (fp64), iteration-invariant; feature-major chunks
    XB = (x.reshape(B, F0).astype(np.float64) @ MB64).astype(np.float32)
    XBT = _chunked(np.ascontiguousarray(XB.T), K1)            # [K1, P, B]

    # feature-major (transposed) activations, batch sharded over cores
    h1T = _chunked(_round_fp32r(h1.reshape(B, F1).T), K1)     # [K1, P, B]
    h2T = _chunked(_round_fp32r(h2.reshape(B, F2).T), K2)     # [K2, P, B]

    in_maps = []
    for c in range(N_CORES):
        csl = slice(c * B_CORE, (c + 1) * B_CORE)
        in_maps.append(
            {
                "XBT": np.ascontiguousarray(XBT[:, :, csl]),
                "h1T": np.ascontiguousarray(h1T[:, :, csl]),
                "h2T": np.ascontiguousarray(h2T[:, :, csl]),
                "MAB": MAB_h,
                "MC": MC_h,
                "MD": MD_h,
            }
        )

    nc = _get_nc()
    kw = {}
    if _trace:
        kw = dict(trace=True, trace_cores=[0])
    out = run_bass_kernel_spmd(nc, in_maps, core_ids=list(range(N_CORES)), **kw)

    h1_out = np.empty((B, F1), np.float32)
    h2_out = np.empty((B, F2), np.float32)
    for c, res in enumerate(out.results):
        csl = slice(c * B_CORE, (c + 1) * B_CORE)
        h1_out[csl] = res["h1o"].reshape(F1, B_CORE).T
        h2_out[csl] = res["h2o"].reshape(F2, B_CORE).T

    h1_out = h1_out.reshape(B, D1, NB)
    h2_out = h2_out.reshape(B, D2, NB)
    if _trace:
        kernel.last_exec_time_ns = out.exec_time_ns
        kernel.last_results = out
    return (x.reshape(B, D0, NB), h1_out, h2_out)


# revision 25
# speedup vs baseline: 1.0429x; 1.0429x over previous
"""Trainium2 Bass kernel for the CliffordPC problem.

Self-contained: takes FULL inputs, shards batch across 8 NeuronCores,
runs all 20 predictive-coding iterations on-chip, returns FULL outputs.

Math. The geometric-product "linear layer"
    out[n,i,c] = sum_{j,a,b} x[n,j,a] * W[i,j,b] * C[a,b,c]
with the Cl(3,0) Cayley table C (one nonzero per (a,b): C[a,b,a^b]=s(a,b))
is exactly a dense matmul over flattened (feature, blade) axes:
    out[n, i*8+c] = sum_{j,a} x[n, j*8+a] * M[(j,a),(i,c)],
    M[(j,a),(i,c)] = s(a, a^c) * W[i, j, a^c].
All operators are precomputed on the host as dense matrices in float32r
(fp32 with mantissa RNE-rounded to 11 bits -> 4x matmul throughput on the
PE array at free-dim >= 256, fp32-exact accumulation in PSUM).

Key fusion: the layer-0 error e0 = x - h1@MA only feeds g1 = e0@MB, so
    g1 = x@MB - h1@(MA@MB) = XB - h1@MAB
where XB = x@MB is iteration-invariant (x never changes; computed once on
chip) and MAB = MA@MB is precomputed on the host. This cuts the dominant
per-iteration work from two 2048-wide matmuls to one 1024-wide matmul
(~2.4x FLOP reduction).
"""

import sys
import os

for _p in ("/opt/trn_rl_repo",):
    if _p not in sys.path and os.path.isdir(_p):
        sys.path.insert(0, _p)

import numpy as np
from contextlib import ExitStack

import concourse.bass as bass
import concourse.tile as tile
from concourse import bacc, mybir
from concourse.bass_utils import run_bass_kernel_spmd

# ---- problem constants (hardcoded per contract) ----
B = 4096
D0, D1, D2 = 256, 128, 64       # layer dims
NB = 8                           # blades (Cl(3))
F0, F1, F2 = D0 * NB, D1 * NB, D2 * NB   # 2048, 1024, 512
N_ITER = 20
ALPHA = 0.01
N_CORES = 8
B_CORE = B // N_CORES            # 512
NT = 512                         # batch-tile width == B_CORE (one tile/core)
P = 128

K0, K1, K2 = F0 // P, F1 // P, F2 // P   # 16, 8, 4 partition-chunks

f32 = mybir.dt.float32
f32r = mybir.dt.float32r


# ---------------- host-side math ----------------

def _cayley_sign():
    """s[a,b] for Cl(3,0,0) with metric (1,1,1): e_a e_b = s[a,b] e_{a^b}."""
    s = np.zeros((NB, NB), np.float32)
    for a in range(NB):
        for b in range(NB):
            cnt = 0
            aa = a >> 1
            while aa:
                cnt += bin(aa & b).count("1")
                aa >>= 1
            s[a, b] = -1.0 if (cnt & 1) else 1.0
    return s


def _rev_signs():
    k = np.array([bin(i).count("1") for i in range(NB)])
    return ((-1.0) ** (k * (k - 1) // 2)).astype(np.float32)


def _build_mats(W1, W2):
    """Expanded dense matrices for the four geometric-product operators."""
    S = _cayley_sign()                       # [a,b]
    REV = _rev_signs()                       # [b]
    a_idx = np.arange(NB)
    c_idx = np.arange(NB)
    Bx = a_idx[:, None] ^ c_idx[None, :]     # b = a^c, [a,c]
    Sac = S[a_idx[:, None], Bx]              # s(a, a^c), [a,c]
    SacR = Sac * REV[Bx]                     # s(a, a^c)*REV[a^c], [a,c]

    # MA[(j,a),(i,c)] = Sac[a,c] * W1[i,j,a^c]         -> [F1, F0]
    W1g = W1[:, :, Bx]                       # [i,j,a,c]
    MA = (Sac[None, None] * W1g).transpose(1, 2, 0, 3).reshape(F1, F0)

    # MB[(j,a),(i,c)] = SacR[a,c] * W1[j,i,a^c]        -> [F0, F1]
    MB = (SacR[None, None] * W1g).transpose(0, 2, 1, 3).reshape(F0, F1)

    # MC[(j,a),(i,c)] = Sac[a,c] * W2[i,j,a^c]         -> [F2, F1]
    W2g = W2[:, :, Bx]                       # [i,j,a,c]
    MC = (Sac[None, None] * W2g).transpose(1, 2, 0, 3).reshape(F2, F1)

    # MD[(j,a),(i,c)] = SacR[a,c] * W2[j,i,a^c]        -> [F1, F2]
    MD = (SacR[None, None] * W2g).transpose(0, 2, 1, 3).reshape(F1, F2)

    return MA, MB, MC, MD


def _round_fp32r(a):
    """RNE-round fp32 to fp32r (drop 12 low mantissa bits) - matches HW cast."""
    b = np.ascontiguousarray(a, dtype=np.float32).view(np.uint32)
    r = b + np.uint32(0x7FF) + ((b >> np.uint32(12)) & np.uint32(1))
    r &= np.uint32(0xFFFFF000)
    return r.view(np.float32)


def _chunked(mat, k_tiles):
    """[K, N] row-major -> [k_tiles, P, N] (k-chunk, partition, cols)."""
    K, N = mat.shape
    assert K == k_tiles * P
    return np.ascontiguousarray(mat.reshape(k_tiles, P, N))


# ---------------- device kernel ----------------

def _build_bass():
    n_iter = int(os.environ.get("KERNEL_N_ITER", N_ITER))
    nc = bacc.Bacc("TRN2", target_bir_lowering=False, debug=False)

    XB_d = nc.dram_tensor("XBT", [K1, P, B_CORE], f32, kind="ExternalInput").ap()
    h1_d = nc.dram_tensor("h1T", [K1, P, B_CORE], f32r, kind="ExternalInput").ap()
    h2_d = nc.dram_tensor("h2T", [K2, P, B_CORE], f32r, kind="ExternalInput").ap()
    MAB_d = nc.dram_tensor("MAB", [K1, P, F1], f32r, kind="ExternalInput").ap()
    MC_d = nc.dram_tensor("MC", [K2, P, F1], f32r, kind="ExternalInput").ap()
    MD_d = nc.dram_tensor("MD", [K1, P, F2], f32r, kind="ExternalInput").ap()
    h1o_d = nc.dram_tensor("h1o", [K1, P, B_CORE], f32, kind="ExternalOutput").ap()
    h2o_d = nc.dram_tensor("h2o", [K2, P, B_CORE], f32, kind="ExternalOutput").ap()
    debug_taps = os.environ.get("KERNEL_DEBUG_TAPS") == "1"
    if debug_taps:
        sdbg_d = nc.dram_tensor("sdbg", [K1, P, B_CORE], f32, kind="ExternalOutput").ap()
        tdbg_d = nc.dram_tensor("tdbg", [K1, P, B_CORE], f32, kind="ExternalOutput").ap()

    AMIN, AMAX = mybir.AluOpType.min, mybir.AluOpType.max
    TW = 256          # per-batch-tile width; two tiles are interleaved so
    NTL = 2           # one tile's matmuls hide the other's vector chains

    with tile.TileContext(nc) as tc, ExitStack() as ctx:
        wpool = ctx.enter_context(tc.tile_pool(name="weights", bufs=1))
        state = ctx.enter_context(tc.tile_pool(name="state", bufs=1))
        tpool = ctx.enter_context(tc.tile_pool(name="tmps", bufs=3))
        # one accumulation group per PSUM bank (HW start=True clears the
        # whole bank's has_written bits): 3 + 3 + 2 = 8 banks
        ps_acc = ctx.enter_context(tc.tile_pool(name="ps_acc", bufs=3, space="PSUM"))
        ps_s = ctx.enter_context(tc.tile_pool(name="ps_s", bufs=3, space="PSUM"))
        ps_g2 = ctx.enter_context(tc.tile_pool(name="ps_g2", bufs=2, space="PSUM"))

        # resident weights
        MAB_s = wpool.tile([P, K1, F1], f32r, tag="MAB")
        MC_s = wpool.tile([P, K2, F1], f32r, tag="MC")
        MD_s = wpool.tile([P, K1, F2], f32r, tag="MD")

        # per-batch-tile states: fp32 masters + f32r matmul copies
        h1m = [state.tile([P, K1, TW], f32, tag=f"h1m{t}", name=f"h1m{t}") for t in range(NTL)]
        h1r = [state.tile([P, K1, TW], f32r, tag=f"h1r{t}", name=f"h1r{t}") for t in range(NTL)]
        h2m = [state.tile([P, K2, TW], f32, tag=f"h2m{t}", name=f"h2m{t}") for t in range(NTL)]
        h2r = [state.tile([P, K2, TW], f32r, tag=f"h2r{t}", name=f"h2r{t}") for t in range(NTL)]
        m_t = [state.tile([P, K1, TW], f32, tag=f"m{t}", name=f"m{t}") for t in range(NTL)]
        XB_t = [state.tile([P, K1, TW], f32, tag=f"XB{t}", name=f"XB{t}") for t in range(NTL)]
        e1_t = [state.tile([P, K1, TW], f32r, tag=f"e1{t}", name=f"e1{t}") for t in range(NTL)]

        def bsl(t):
            return bass.ts(t, TW)

        # Cold-start DMA across queues; A(0) needs h2r+MC first.
        for t in range(NTL):
            nc.sync.dma_start(h2r[t][:], h2_d.rearrange("k p b -> p k b")[:, :, bsl(t)])
            nc.sync.dma_start(
                h2m[t][:], h2_d.bitcast(f32).rearrange("k p b -> p k b")[:, :, bsl(t)]
            )
        nc.gpsimd.dma_start(MC_s[:], MC_d.rearrange("k p n -> p k n"))
        for t in range(NTL):
            nc.sync.dma_start(h1r[t][:], h1_d.rearrange("k p b -> p k b")[:, :, bsl(t)])
            nc.sync.dma_start(
                h1m[t][:], h1_d.bitcast(f32).rearrange("k p b -> p k b")[:, :, bsl(t)]
            )
        for k in range(K1):
            nc.scalar.dma_start(
                MAB_s[:, k, :], MAB_d.rearrange("k p n -> p k n")[:, k, :]
            )
        for t in range(NTL):
            nc.sync.dma_start(XB_t[t][:], XB_d.rearrange("k p b -> p k b")[:, :, bsl(t)])
        nc.gpsimd.dma_start(MD_s[:], MD_d.rearrange("k p n -> p k n"))

        def phase_A(t, it):
            # p2 = MC.h2 ; m = h1 - p2 (fused eviction)
            for g in range(K1):
                ps = ps_acc.tile([P, TW], f32, tag="acc", name=f"p2_{it}_{t}_{g}")
                for k in range(K2):
                    nc.tensor.matmul(
                        ps[:],
                        MC_s[:, k, bass.ts(g, P)],
                        h2r[t][:, k, :],
                        start=(k == 0),
                        stop=(k == K2 - 1),
                    )
                nc.vector.tensor_sub(m_t[t][:, g, :], h1m[t][:, g, :], ps[:])

        for t in range(NTL):
            phase_A(t, 0)

        for it in range(n_iter):
            # ---- C1 (tile-interleaved): S = MAB.h1 ;
            #      c2 = ALPHA*clip((m - XB) + S) ; e1 = m - c2 ;
            #      h1 -= c2 ; h1r = round(h1) ----
            for g in range(K1):
                for t in range(NTL):
                    sps = ps_s.tile([P, TW], f32, tag="s", name=f"s_{it}_{t}_{g}")
                    for k in range(K1):
                        nc.tensor.matmul(
                            sps[:],
                            MAB_s[:, k, bass.ts(g, P)],
                            h1r[t][:, k, :],
                            start=(k == 0),
                            stop=(k == K1 - 1),
                        )
                    tt = tpool.tile([P, TW], f32, tag="t")
                    nc.vector.tensor_sub(tt[:], m_t[t][:, g, :], XB_t[t][:, g, :])
                    if debug_taps and it == 0:
                        sdb = tpool.tile([P, TW], f32, tag="sdb")
                        nc.vector.tensor_copy(sdb[:], sps[:])
                        nc.sync.dma_start(sdbg_d.rearrange("k p b -> p k b")[:, g, bsl(t)], sdb[:])
                    nc.vector.tensor_add(tt[:], tt[:], sps[:])
                    if debug_taps and it == 0:
                        nc.sync.dma_start(tdbg_d.rearrange("k p b -> p k b")[:, g, bsl(t)], tt[:])
                    t2 = tpool.tile([P, TW], f32, tag="t2")
                    # c2 = ALPHA*clip(t,-1,1) == clip(ALPHA*t, -ALPHA, ALPHA),
                    # kept on DVE: the ACT scale-multiply is internally
                    # low-precision and biased the h1 trajectory
                    nc.vector.tensor_scalar(t2[:], tt[:], ALPHA, ALPHA, mybir.AluOpType.mult, AMIN)
                    nc.vector.tensor_scalar_max(t2[:], t2[:], -ALPHA)
                    nc.vector.tensor_sub(e1_t[t][:, g, :], m_t[t][:, g, :], t2[:])
                    # master update off the DVE critical path
                    nc.gpsimd.tensor_sub(h1m[t][:, g, :], h1m[t][:, g, :], t2[:])

            # refresh the f32r matmul copies only after ALL S matmuls of
            # this iteration have read the old h1r (Jacobi, matching the
            # reference; updating in-loop would leak new chunks into later
            # S contractions)
            for t in range(NTL):
                for g in range(K1):
                    nc.scalar.mul(h1r[t][:, g, :], h1m[t][:, g, :], 1.0)

            # ---- C2+D per tile: g2 = MD.e1 (q-outer, one bank per group) ;
            #      h2 += ALPHA*clip(g2) ; h2r = round(h2) ----
            for t in range(NTL):
                for q in range(K2):
                    g2ps = ps_g2.tile([P, TW], f32, tag="g2", name=f"g2_{it}_{t}_{q}")
                    for g in range(K1):
                        nc.tensor.matmul(
                            g2ps[:],
                            MD_s[:, g, bass.ts(q, P)],
                            e1_t[t][:, g, :],
                            start=(g == 0),
                            stop=(g == K1 - 1),
                        )
                    u = tpool.tile([P, TW], f32, tag="t", name=f"u_{it}_{t}_{q}")
                    nc.vector.tensor_scalar(u[:], g2ps[:], 1.0, -1.0, AMIN, AMAX)
                    nc.scalar.mul(u[:], u[:], ALPHA)
                    nc.gpsimd.tensor_add(h2m[t][:, q, :], h2m[t][:, q, :], u[:])
                    nc.scalar.mul(h2r[t][:, q, :], h2m[t][:, q, :], 1.0)

            if it + 1 < n_iter:
                for t in range(NTL):
                    phase_A(t, it + 1)

        for t in range(NTL):
            nc.gpsimd.dma_start(
                h1o_d.rearrange("k p b -> p k b")[:, :, bsl(t)], h1m[t][:]
            )
            nc.gpsimd.dma_start(
                h2o_d.rearrange("k p b -> p k b")[:, :, bsl(t)], h2m[t][:]
            )

    nc.compile()
    return nc


_NC_CACHE = None


def _get_nc():
    global _NC_CACHE
    if _NC_CACHE is None:
        _NC_CACHE = _build_bass()
    return _NC_CACHE


def kernel(x, W1, W2, h1, h2, _trace=False):
    x = np.asarray(x, dtype=np.float32)
    W1 = np.asarray(W1, dtype=np.float32)
    W2 = np.asarray(W2, dtype=np.float32)
    h1 = np.asarray(h1, dtype=np.float32)
    h2 = np.asarray(h2, dtype=np.float32)

    MA, MB, MC, MD = _build_mats(W1, W2)
    MB64 = MB.astype(np.float64)
    MAB = (MA.astype(np.float64) @ MB64).astype(np.float32)
    MAB_h = _chunked(_round_fp32r(MAB), K1)
    MC_h = _chunked(_round_fp32r(MC), K2)
    MD_h = _chunked(_round_fp32r(MD), K1)
    # XB = x @ MB on host (fp64), iteration-invariant; feature-major chunks
    XB = (x.reshape(B, F0).astype(np.float64) @ MB64).astype(np.float32)
    XBT = _chunked(np.ascontiguousarray(XB.T), K1)            # [K1, P, B]

    # feature-major (transposed) activations, batch sharded over cores
    h1T = _chunked(_round_fp32r(h1.reshape(B, F1).T), K1)     # [K1, P, B]
    h2T = _chunked(_round_fp32r(h2.reshape(B, F2).T), K2)     # [K2, P, B]

    in_maps = []
    for c in range(N_CORES):
        csl = slice(c * B_CORE, (c + 1) * B_CORE)
        in_maps.append(
            {
                "XBT": np.ascontiguousarray(XBT[:, :, csl]),
                "h1T": np.ascontiguousarray(h1T[:, :, csl]),
                "h2T": np.ascontiguousarray(h2T[:, :, csl]),
                "MAB": MAB_h,
                "MC": MC_h,
                "MD": MD_h,
            }
        )

    nc = _get_nc()
    kw = {}
    if _trace:
        kw = dict(trace=True, trace_cores=[0])
    out = run_bass_kernel_spmd(nc, in_maps, core_ids=list(range(N_CORES)), **kw)

    h1_out = np.empty((B, F1), np.float32)
    h2_out = np.empty((B, F2), np.float32)
    for c, res in enumerate(out.results):
        csl = slice(c * B_CORE, (c + 1) * B_CORE)
        h1_out[csl] = res["h1o"].reshape(F1, B_CORE).T
        h2_out[csl] = res["h2o"].reshape(F2, B_CORE).T

    h1_out = h1_out.reshape(B, D1, NB)
    h2_out = h2_out.reshape(B, D2, NB)
    kernel.last_results = out
    if _trace:
        kernel.last_exec_time_ns = out.exec_time_ns
    return (x.reshape(B, D0, NB), h1_out, h2_out)


# revision 26
# speedup vs baseline: 1.0815x; 1.0370x over previous
"""Trainium2 Bass kernel for the CliffordPC problem.

Self-contained: takes FULL inputs, shards batch across 8 NeuronCores,
runs all 20 predictive-coding iterations on-chip, returns FULL outputs.

Math. The geometric-product "linear layer"
    out[n,i,c] = sum_{j,a,b} x[n,j,a] * W[i,j,b] * C[a,b,c]
with the Cl(3,0) Cayley table C (one nonzero per (a,b): C[a,b,a^b]=s(a,b))
is exactly a dense matmul over flattened (feature, blade) axes:
    out[n, i*8+c] = sum_{j,a} x[n, j*8+a] * M[(j,a),(i,c)],
    M[(j,a),(i,c)] = s(a, a^c) * W[i, j, a^c].
All operators are precomputed on the host as dense matrices in float32r
(fp32 with mantissa RNE-rounded to 11 bits -> 4x matmul throughput on the
PE array at free-dim >= 256, fp32-exact accumulation in PSUM).

Key fusion: the layer-0 error e0 = x - h1@MA only feeds g1 = e0@MB, so
    g1 = x@MB - h1@(MA@MB) = XB - h1@MAB
where XB = x@MB is iteration-invariant (x never changes; computed once on
chip) and MAB = MA@MB is precomputed on the host. This cuts the dominant
per-iteration work from two 2048-wide matmuls to one 1024-wide matmul
(~2.4x FLOP reduction).
"""

import sys
import os

for _p in ("/opt/trn_rl_repo",):
    if _p not in sys.path and os.path.isdir(_p):
        sys.path.insert(0, _p)

import numpy as np
from contextlib import ExitStack

import concourse.bass as bass
import concourse.tile as tile
from concourse import bacc, mybir
from concourse.bass_utils import run_bass_kernel_spmd

# ---- problem constants (hardcoded per contract) ----
B = 4096
D0, D1, D2 = 256, 128, 64       # layer dims
NB = 8                           # blades (Cl(3))
F0, F1, F2 = D0 * NB, D1 * NB, D2 * NB   # 2048, 1024, 512
N_ITER = 20
ALPHA = 0.01
N_CORES = 8
B_CORE = B // N_CORES            # 512
NT = 512                         # batch-tile width == B_CORE (one tile/core)
P = 128

K0, K1, K2 = F0 // P, F1 // P, F2 // P   # 16, 8, 4 partition-chunks

f32 = mybir.dt.float32
f32r = mybir.dt.float32r


# ---------------- host-side math ----------------

def _cayley_sign():
    """s[a,b] for Cl(3,0,0) with metric (1,1,1): e_a e_b = s[a,b] e_{a^b}."""
    s = np.zeros((NB, NB), np.float32)
    for a in range(NB):
        for b in range(NB):
            cnt = 0
            aa = a >> 1
            while aa:
                cnt += bin(aa & b).count("1")
                aa >>= 1
            s[a, b] = -1.0 if (cnt & 1) else 1.0
    return s


def _rev_signs():
    k = np.array([bin(i).count("1") for i in range(NB)])
    return ((-1.0) ** (k * (k - 1) // 2)).astype(np.float32)


def _build_mats(W1, W2):
    """Expanded dense matrices for the four geometric-product operators."""
    S = _cayley_sign()                       # [a,b]
    REV = _rev_signs()                       # [b]
    a_idx = np.arange(NB)
    c_idx = np.arange(NB)
    Bx = a_idx[:, None] ^ c_idx[None, :]     # b = a^c, [a,c]
    Sac = S[a_idx[:, None], Bx]              # s(a, a^c), [a,c]
    SacR = Sac * REV[Bx]                     # s(a, a^c)*REV[a^c], [a,c]

    # MA[(j,a),(i,c)] = Sac[a,c] * W1[i,j,a^c]         -> [F1, F0]
    W1g = W1[:, :, Bx]                       # [i,j,a,c]
    MA = (Sac[None, None] * W1g).transpose(1, 2, 0, 3).reshape(F1, F0)

    # MB[(j,a),(i,c)] = SacR[a,c] * W1[j,i,a^c]        -> [F0, F1]
    MB = (SacR[None, None] * W1g).transpose(0, 2, 1, 3).reshape(F0, F1)

    # MC[(j,a),(i,c)] = Sac[a,c] * W2[i,j,a^c]         -> [F2, F1]
    W2g = W2[:, :, Bx]                       # [i,j,a,c]
    MC = (Sac[None, None] * W2g).transpose(1, 2, 0, 3).reshape(F2, F1)

    # MD[(j,a),(i,c)] = SacR[a,c] * W2[j,i,a^c]        -> [F1, F2]
    MD = (SacR[None, None] * W2g).transpose(0, 2, 1, 3).reshape(F1, F2)

    return MA, MB, MC, MD


def _round_fp32r(a):
    """RNE-round fp32 to fp32r (drop 12 low mantissa bits) - matches HW cast."""
    b = np.ascontiguousarray(a, dtype=np.float32).view(np.uint32)
    r = b + np.uint32(0x7FF) + ((b >> np.uint32(12)) & np.uint32(1))
    r &= np.uint32(0xFFFFF000)
    return r.view(np.float32)


def _chunked(mat, k_tiles):
    """[K, N] row-major -> [k_tiles, P, N] (k-chunk, partition, cols)."""
    K, N = mat.shape
    assert K == k_tiles * P
    return np.ascontiguousarray(mat.reshape(k_tiles, P, N))


# ---------------- device kernel ----------------

def _build_bass():
    n_iter = int(os.environ.get("KERNEL_N_ITER", N_ITER))
    nc = bacc.Bacc("TRN2", target_bir_lowering=False, debug=False)

    XB_d = nc.dram_tensor("XBT", [K1, P, B_CORE], f32, kind="ExternalInput").ap()
    h1_d = nc.dram_tensor("h1T", [K1, P, B_CORE], f32r, kind="ExternalInput").ap()
    h2_d = nc.dram_tensor("h2T", [K2, P, B_CORE], f32r, kind="ExternalInput").ap()
    MAB_d = nc.dram_tensor("MAB", [K1, P, F1], f32r, kind="ExternalInput").ap()
    MC_d = nc.dram_tensor("MC", [K2, P, F1], f32r, kind="ExternalInput").ap()
    MD_d = nc.dram_tensor("MD", [K1, P, F2], f32r, kind="ExternalInput").ap()
    h1o_d = nc.dram_tensor("h1o", [K1, P, B_CORE], f32, kind="ExternalOutput").ap()
    h2o_d = nc.dram_tensor("h2o", [K2, P, B_CORE], f32, kind="ExternalOutput").ap()
    debug_taps = os.environ.get("KERNEL_DEBUG_TAPS") == "1"
    if debug_taps:
        sdbg_d = nc.dram_tensor("sdbg", [K1, P, B_CORE], f32, kind="ExternalOutput").ap()
        tdbg_d = nc.dram_tensor("tdbg", [K1, P, B_CORE], f32, kind="ExternalOutput").ap()

    AMIN, AMAX = mybir.AluOpType.min, mybir.AluOpType.max
    TW = 256          # per-batch-tile width; two tiles are interleaved so
    NTL = 2           # one tile's matmuls hide the other's vector chains

    with tile.TileContext(nc) as tc, ExitStack() as ctx:
        wpool = ctx.enter_context(tc.tile_pool(name="weights", bufs=1))
        state = ctx.enter_context(tc.tile_pool(name="state", bufs=1))
        tpool = ctx.enter_context(tc.tile_pool(name="tmps", bufs=3))
        # one accumulation group per PSUM bank (HW start=True clears the
        # whole bank's has_written bits): 3 + 3 + 2 = 8 banks
        ps_acc = ctx.enter_context(tc.tile_pool(name="ps_acc", bufs=3, space="PSUM"))
        ps_s = ctx.enter_context(tc.tile_pool(name="ps_s", bufs=3, space="PSUM"))
        ps_g2 = ctx.enter_context(tc.tile_pool(name="ps_g2", bufs=2, space="PSUM"))

        # resident weights
        MAB_s = wpool.tile([P, K1, F1], f32r, tag="MAB")
        MC_s = wpool.tile([P, K2, F1], f32r, tag="MC")
        MD_s = wpool.tile([P, K1, F2], f32r, tag="MD")

        # per-batch-tile states: fp32 masters + f32r matmul copies
        h1m = [state.tile([P, K1, TW], f32, tag=f"h1m{t}", name=f"h1m{t}") for t in range(NTL)]
        h1r = [state.tile([P, K1, TW], f32r, tag=f"h1r{t}", name=f"h1r{t}") for t in range(NTL)]
        h2m = [state.tile([P, K2, TW], f32, tag=f"h2m{t}", name=f"h2m{t}") for t in range(NTL)]
        h2r = [state.tile([P, K2, TW], f32r, tag=f"h2r{t}", name=f"h2r{t}") for t in range(NTL)]
        m_t = [state.tile([P, K1, TW], f32, tag=f"m{t}", name=f"m{t}") for t in range(NTL)]
        XB_t = [state.tile([P, K1, TW], f32, tag=f"XB{t}", name=f"XB{t}") for t in range(NTL)]
        e1_t = [state.tile([P, K1, TW], f32r, tag=f"e1{t}", name=f"e1{t}") for t in range(NTL)]

        def bsl(t):
            return bass.ts(t, TW)

        # Cold-start DMA across queues; A(0) needs h2r+MC first.
        for t in range(NTL):
            nc.sync.dma_start(h2r[t][:], h2_d.rearrange("k p b -> p k b")[:, :, bsl(t)])
            nc.sync.dma_start(
                h2m[t][:], h2_d.bitcast(f32).rearrange("k p b -> p k b")[:, :, bsl(t)]
            )
        nc.gpsimd.dma_start(MC_s[:], MC_d.rearrange("k p n -> p k n"))
        for t in range(NTL):
            nc.sync.dma_start(h1r[t][:], h1_d.rearrange("k p b -> p k b")[:, :, bsl(t)])
            nc.sync.dma_start(
                h1m[t][:], h1_d.bitcast(f32).rearrange("k p b -> p k b")[:, :, bsl(t)]
            )
        for k in range(K1):
            q = nc.scalar if k % 2 == 0 else nc.gpsimd
            q.dma_start(MAB_s[:, k, :], MAB_d.rearrange("k p n -> p k n")[:, k, :])
        for t in range(NTL):
            nc.sync.dma_start(XB_t[t][:], XB_d.rearrange("k p b -> p k b")[:, :, bsl(t)])
        nc.gpsimd.dma_start(MD_s[:], MD_d.rearrange("k p n -> p k n"))

        def phase_A(t, it):
            # p2 = MC.h2 ; m = h1 - p2 (fused eviction)
            for g in range(K1):
                ps = ps_acc.tile([P, TW], f32, tag="acc", name=f"p2_{it}_{t}_{g}")
                for k in range(K2):
                    nc.tensor.matmul(
                        ps[:],
                        MC_s[:, k, bass.ts(g, P)],
                        h2r[t][:, k, :],
                        start=(k == 0),
                        stop=(k == K2 - 1),
                    )
                nc.vector.tensor_sub(m_t[t][:, g, :], h1m[t][:, g, :], ps[:])

        for t in range(NTL):
            phase_A(t, 0)

        for it in range(n_iter):
            # ---- C1 (tile-interleaved): S = MAB.h1 ;
            #      c2 = ALPHA*clip((m - XB) + S) ; e1 = m - c2 ;
            #      h1 -= c2 ; h1r = round(h1) ----
            for g in range(K1):
                for t in range(NTL):
                    sps = ps_s.tile([P, TW], f32, tag="s", name=f"s_{it}_{t}_{g}")
                    for k in range(K1):
                        nc.tensor.matmul(
                            sps[:],
                            MAB_s[:, k, bass.ts(g, P)],
                            h1r[t][:, k, :],
                            start=(k == 0),
                            stop=(k == K1 - 1),
                        )
                    tt = tpool.tile([P, TW], f32, tag="t")
                    nc.gpsimd.tensor_sub(tt[:], m_t[t][:, g, :], XB_t[t][:, g, :])
                    if debug_taps and it == 0:
                        sdb = tpool.tile([P, TW], f32, tag="sdb")
                        nc.vector.tensor_copy(sdb[:], sps[:])
                        nc.sync.dma_start(sdbg_d.rearrange("k p b -> p k b")[:, g, bsl(t)], sdb[:])
                    nc.vector.tensor_add(tt[:], tt[:], sps[:])
                    if debug_taps and it == 0:
                        nc.sync.dma_start(tdbg_d.rearrange("k p b -> p k b")[:, g, bsl(t)], tt[:])
                    t2 = tpool.tile([P, TW], f32, tag="t2")
                    nc.vector.tensor_scalar(t2[:], tt[:], 1.0, -1.0, AMIN, AMAX)
                    nc.scalar.mul(t2[:], t2[:], ALPHA)
                    nc.vector.tensor_sub(e1_t[t][:, g, :], m_t[t][:, g, :], t2[:])
                    # master update off the DVE critical path
                    nc.gpsimd.tensor_sub(h1m[t][:, g, :], h1m[t][:, g, :], t2[:])

            # refresh the f32r matmul copies only after ALL S matmuls of
            # this iteration have read the old h1r (Jacobi, matching the
            # reference; updating in-loop would leak new chunks into later
            # S contractions)
            for t in range(NTL):
                for g in range(K1):
                    nc.scalar.mul(h1r[t][:, g, :], h1m[t][:, g, :], 1.0)

            # ---- C2+D per tile: g2 = MD.e1 (q-outer, one bank per group) ;
            #      h2 += ALPHA*clip(g2) ; h2r = round(h2) ----
            for t in range(NTL):
                for q in range(K2):
                    g2ps = ps_g2.tile([P, TW], f32, tag="g2", name=f"g2_{it}_{t}_{q}")
                    for g in range(K1):
                        nc.tensor.matmul(
                            g2ps[:],
                            MD_s[:, g, bass.ts(q, P)],
                            e1_t[t][:, g, :],
                            start=(g == 0),
                            stop=(g == K1 - 1),
                        )
                    u = tpool.tile([P, TW], f32, tag="t", name=f"u_{it}_{t}_{q}")
                    nc.vector.tensor_scalar(u[:], g2ps[:], 1.0, -1.0, AMIN, AMAX)
                    nc.scalar.mul(u[:], u[:], ALPHA)
                    nc.gpsimd.tensor_add(h2m[t][:, q, :], h2m[t][:, q, :], u[:])
                    nc.scalar.mul(h2r[t][:, q, :], h2m[t][:, q, :], 1.0)

            if it + 1 < n_iter:
                for t in range(NTL):
                    phase_A(t, it + 1)
            else:
                for t in range(NTL):
                    nc.sync.dma_start(
                        h1o_d.rearrange("k p b -> p k b")[:, :, bsl(t)], h1m[t][:]
                    )
                    nc.scalar.dma_start(
                        h2o_d.rearrange("k p b -> p k b")[:, :, bsl(t)], h2m[t][:]
                    )

    nc.compile()
    return nc


_NC_CACHE = None


def _get_nc():
    global _NC_CACHE
    if _NC_CACHE is None:
        _NC_CACHE = _build_bass()
    return _NC_CACHE


def kernel(x, W1, W2, h1, h2, _trace=False):
    x = np.asarray(x, dtype=np.float32)
    W1 = np.asarray(W1, dtype=np.float32)
    W2 = np.asarray(W2, dtype=np.float32)
    h1 = np.asarray(h1, dtype=np.float32)
    h2 = np.asarray(h2, dtype=np.float32)

    MA, MB, MC, MD = _build_mats(W1, W2)
    MB64 = MB.astype(np.float64)
    MAB = (MA.astype(np.float64) @ MB64).astype(np.float32)
    MAB_h = _chunked(_round_fp32r(MAB), K1)
    MC_h = _chunked(_round_fp32r(MC), K2)
    MD_h = _chunked(_round_fp32r(MD), K1)
    # XB = x @ MB on host (fp64), iteration-invariant; feature-major chunks
    XB = (x.reshape(B, F0).astype(np.float64) @ MB64).astype(np.float32)
    XBT = _chunked(np.ascontiguousarray(XB.T), K1)            # [K1, P, B]

    # feature-major (transposed) activations, batch sharded over cores
    h1T = _chunked(_round_fp32r(h1.reshape(B, F1).T), K1)     # [K1, P, B]
    h2T = _chunked(_round_fp32r(h2.reshape(B, F2).T), K2)     # [K2, P, B]

    in_maps = []
    for c in range(N_CORES):
        csl = slice(c * B_CORE, (c + 1) * B_CORE)
        in_maps.append(
            {
                "XBT": np.ascontiguousarray(XBT[:, :, csl]),
                "h1T": np.ascontiguousarray(h1T[:, :, csl]),
                "h2T": np.ascontiguousarray(h2T[:, :, csl]),
                "MAB": MAB_h,
                "MC": MC_h,
                "MD": MD_h,
            }
        )

    nc = _get_nc()
    kw = {}
    if _trace:
        kw = dict(trace=True, trace_cores=[0])
    out = run_bass_kernel_spmd(nc, in_maps, core_ids=list(range(N_CORES)), **kw)

    h1_out = np.empty((B, F1), np.float32)
    h2_out = np.empty((B, F2), np.float32)
    for c, res in enumerate(out.results):
        csl = slice(c * B_CORE, (c + 1) * B_CORE)
        h1_out[csl] = res["h1o"].reshape(F1, B_CORE).T
        h2_out[csl] = res["h2o"].reshape(F2, B_CORE).T

    h1_out = h1_out.reshape(B, D1, NB)
    h2_out = h2_out.reshape(B, D2, NB)
    kernel.last_results = out
    if _trace:
        kernel.last_exec_time_ns = out.exec_time_ns
    return (x.reshape(B, D0, NB), h1_out, h2_out)
